# revision 3
# baseline (speedup 1.0000x reference)
"""Self-contained TRN2 Bass kernel for nn_DQN (snake feature extraction + tiny MLP).

kernel(**inputs) takes FULL inputs (x [8192,3,32,32] f32, w1, b1, w2, b2),
shards the batch dim across 8 NeuronCores (pure data parallel), runs a
Bass/Tile kernel via run_bass_kernel_spmd, returns the FULL (8192, 3) f32
output.

Device algorithm per core (1024 batches = 8 tiles of 128 partitions):
  Each batch occupies one partition lane; free dim holds [g0 grid | g1 grid]
  (2048 cells). Values: g0 in {-1 food, 0, 1 head, 2 obs}, g1 in {0, 1 prev}.
  Stream:  ab = Square(3*g0-1) in {16,1,4,25} [ACT]; max8+max_index give all
  special cells (<=4 obs, food, head) in one pass [DVE]; prev = sum(c*g1)
  via fused affine_mul_reduce [DVE]. Tiny per-batch stage reconstructs
  coordinates (bit ops on indices), builds the relative direction frame,
  raycasts against the <=4 obstacle coords + walls, rotates diff, applies
  the 5-20-3 MLP. Exact integer arithmetic in f32 throughout the features.
"""

import numpy as np

import concourse.bacc as bacc
import concourse.mybir as mybir
import concourse.tile as tile
from concourse.bass_utils import run_bass_kernel_spmd

F32 = mybir.dt.float32
U16 = mybir.dt.uint16
I32 = mybir.dt.int32
BF16 = mybir.dt.bfloat16
OP = mybir.AluOpType
ACTF = mybir.ActivationFunctionType

N_CORES = 8
B_FULL = 8192
T = 8
B_CORE = 128 * T


def build_dqn(tc, outs, ins, T=8, bf16=True):
    nc = tc.nc
    X = ins["x"]          # [128, T, 1024] f32 DRAM (frame-0 grids)
    G1T = ins["g1t"]      # [128, 8*1024] f32 DRAM (frame-1, [cellp, chunk, b'=p*8+t])
    WALL = ins["wall"]    # [128, 183] f32 DRAM
    OUT = outs["out"]     # [128, T*3] f32 DRAM
    ABDT = BF16 if bf16 else F32

    with (
        tc.tile_pool(name="io", bufs=3) as io,
        tc.tile_pool(name="abp", bufs=3) as abp,
        tc.tile_pool(name="st", bufs=1) as st,
        tc.tile_pool(name="ps", bufs=1, space="PSUM") as ps,
        tc.tile_pool(name="dr", bufs=1, space="DRAM") as dr,
    ):
        WS = st.tile([128, 183], F32, name="WS")
        nc.sync.dma_start(out=WS, in_=WALL)
        BN1 = st.tile([128, 1], F32, tag="BN1", name="BN1")
        nc.vector.memset(BN1[:], -1.0)

        # ---- prev extraction on TensorE ----
        # g1t holds frame-1 transposed: row p = cell (chunk*128+p), col = b'=p_b*8+t.
        # prev_flat(b) = sum_cell cell * g1[b, cell] = matmul(iota_chunk.T @ g1t_chunk).
        NB = 128 * T
        G1S = st.tile([128, 8 * NB], F32, name="G1S")
        nc.sync.dma_start(out=G1S, in_=G1T)
        CIDXI = st.tile([128, 8], I32, name="CIDXI")
        nc.gpsimd.iota(CIDXI[:], pattern=[[128, 8]], base=0, channel_multiplier=1)
        CIDX2 = st.tile([128, 8], F32, name="CIDX2")
        nc.scalar.copy(CIDX2[:], CIDXI[:])
        PM = ps.tile([1, NB], F32, name="PM")
        for j in range(8):
            for h in range(0, NB, 512):
                he = min(h + 512, NB)
                nc.tensor.matmul(
                    PM[:, h:he],
                    lhsT=CIDX2[:, j : j + 1],
                    rhs=G1S[:, j * NB + h : j * NB + he],
                    start=(j == 0),
                    stop=(j == 7),
                )
        PRS = st.tile([1, NB], F32, name="PRS")
        nc.scalar.copy(PRS[:], PM[:])
        PRD = dr.tile([1, NB], F32, name="PRD")
        nc.sync.dma_start(out=PRD, in_=PRS[:])
        PREVC = st.tile([128, T], F32, name="PREVC")
        nc.sync.dma_start(out=PREVC, in_=PRD[:].rearrange("o (p t) -> (o p) t", p=128))

        VS = st.tile([128, T * 8], ABDT, name="VS")
        IS = st.tile([128, T * 8], U16, name="IS")

        for t in range(T):
            xt = io.tile([128, 1024], F32, tag="xt", name=f"xt{t}")
            nc.sync.dma_start(out=xt, in_=X[:, t, :])
            ab = abp.tile([128, 1024], ABDT, tag="ab", name=f"ab{t}")
            nc.scalar.activation(ab, xt, ACTF.Square, bias=BN1[:], scale=3.0)
            nc.vector.max(out=VS[:, 8 * t : 8 * t + 8], in_=ab)
            nc.vector.max_index(
                out=IS[:, 8 * t : 8 * t + 8],
                in_max=VS[:, 8 * t : 8 * t + 8],
                in_values=ab,
            )

        # ---------------- tiny stage ----------------
        def t8(name, dt=F32):
            return st.tile([128, T * 8], dt, tag=name, name=name)

        def tT(name, k=1, dt=F32):
            return st.tile([128, T * k], dt, tag=name, name=name)

        VT = VS[:].rearrange("p (t s) -> p t s", s=8)

        M25 = st.tile([128, T * 4], F32, tag="M25", name="M25")
        M25T = M25[:].rearrange("p (t s) -> p t s", s=4)
        nc.vector.tensor_scalar(M25T, VT[:, :, 0:4], 25.0, None, op0=OP.is_equal)

        # mask stack [128, T, 2, 8]: 0=food(16), 1=head(4)
        MST = st.tile([128, T * 2 * 8], F32, tag="MST", name="MST")
        MSTT = MST[:].rearrange("p (t m s) -> p t m s", m=2, s=8)
        nc.vector.tensor_scalar(MSTT[:, :, 0, :], VT, 16.0, None, op0=OP.is_equal)
        nc.vector.tensor_scalar(MSTT[:, :, 1, :], VT, 4.0, None, op0=OP.is_equal)

        # row/col per slot from idx bits
        CP = st.tile([128, T * 2 * 8], F32, tag="CP", name="CP")
        CPT = CP[:].rearrange("p (t m s) -> p t m s", m=2, s=8)
        OCCI = t8("OCCI", U16)
        nc.vector.tensor_scalar(OCCI[:], IS[:], 31, None, op0=OP.bitwise_and)
        ORI = t8("ORI", U16)
        nc.vector.tensor_scalar(ORI[:], IS[:], 5, 31, op0=OP.logical_shift_right, op1=OP.bitwise_and)
        nc.vector.tensor_copy(CPT[:, :, 1, :], OCCI[:].rearrange("p (t s) -> p t s", s=8))
        nc.vector.tensor_copy(CPT[:, :, 0, :], ORI[:].rearrange("p (t s) -> p t s", s=8))

        # head/food coords: PROD[t,m,c,s] = CP[t,c,s]*MST[t,m,s] -> reduce s
        PROD = st.tile([128, T * 2 * 2 * 8], F32, tag="PROD", name="PROD")
        PRODT = PROD[:].rearrange("p (t m c s) -> p t m c s", m=2, c=2, s=8)
        nc.vector.tensor_tensor(
            PRODT,
            CPT[:, :, None, :, :].to_broadcast([128, T, 2, 2, 8]),
            MSTT[:, :, :, None, :].to_broadcast([128, T, 2, 2, 8]),
            OP.mult,
        )
        COORD = st.tile([128, T * 2 * 2], F32, tag="COORD", name="COORD")
        COORDT = COORD[:].rearrange("p (t m c) -> p t m c", m=2, c=2)
        nc.vector.reduce_sum(COORDT, PRODT, axis=mybir.AxisListType.X)
        # [:, :, 0, :] = (fr, fc); [:, :, 1, :] = (hr, hc)

        # prev coords from PREVC: u16 bits
        PRU = tT("PRU", 1, U16)
        nc.vector.tensor_copy(PRU[:], PREVC[:])
        PRB = st.tile([128, T * 2], U16, tag="PRB", name="PRB")
        PRBT = PRB[:].rearrange("p (t c) -> p t c", c=2)
        nc.vector.tensor_scalar(PRBT[:, :, 1:2], PRU[:].rearrange("p (t c) -> p t c", c=1), 31, None, op0=OP.bitwise_and)
        nc.vector.tensor_scalar(PRBT[:, :, 0:1], PRU[:].rearrange("p (t c) -> p t c", c=1), 5, 31, op0=OP.logical_shift_right, op1=OP.bitwise_and)
        PREVF = tT("PREVF", 2)
        PREVFT = PREVF[:].rearrange("p (t c) -> p t c", c=2)
        nc.vector.tensor_copy(PREVFT, PRBT)

        # d = head - prev ; diff = food - head
        D2 = tT("D2", 2)
        D2T = D2[:].rearrange("p (t c) -> p t c", c=2)
        nc.vector.tensor_tensor(D2T, COORDT[:, :, 1, :], PREVFT, OP.subtract)
        F2 = tT("F2", 2)
        F2T = F2[:].rearrange("p (t c) -> p t c", c=2)
        nc.vector.tensor_tensor(F2T, COORDT[:, :, 0, :], COORDT[:, :, 1, :], OP.subtract)

        dx = D2T[:, :, 0:1]
        dy = D2T[:, :, 1:2]
        hr = COORDT[:, :, 1, 0:1]
        hc = COORDT[:, :, 1, 1:2]

        FEAT = tT("FEAT", 5)
        FEATT = FEAT[:].rearrange("p (t c) -> p t c", c=5)
        RP = tT("RP", 4)
        RPT = RP[:].rearrange("p (t a b) -> p t a b", a=2, b=2)
        nc.vector.tensor_tensor(
            RPT,
            D2T[:, :, :, None].to_broadcast([128, T, 2, 2]),
            F2T[:, :, None, :].to_broadcast([128, T, 2, 2]),
            OP.mult,
        )
        nc.vector.tensor_tensor(FEATT[:, :, 3:4], RPT[:, :, 0, 0:1], RPT[:, :, 1, 1:2], OP.add)
        nc.vector.tensor_tensor(FEATT[:, :, 4:5], RPT[:, :, 0, 1:2], RPT[:, :, 1, 0:1], OP.subtract)

        # ---- rays (slots 0:4 only — obstacles always sort into the top 4) ----
        def t4(name, dt=F32):
            return st.tile([128, T * 4], dt, tag=name, name=name)

        DR = t4("DR")
        DRT = DR[:].rearrange("p (t s) -> p t s", s=4)
        nc.vector.tensor_tensor(DRT, CPT[:, :, 0, 0:4], hr.to_broadcast([128, T, 4]), OP.subtract)
        DC = t4("DC")
        DCT = DC[:].rearrange("p (t s) -> p t s", s=4)
        nc.vector.tensor_tensor(DCT, CPT[:, :, 1, 0:4], hc.to_broadcast([128, T, 4]), OP.subtract)

        def mk8(name, in0, in1, op):
            tl = t4(name)
            tv = tl[:].rearrange("p (t s) -> p t s", s=4)
            nc.vector.tensor_tensor(tv, in0, in1, op)
            return tv

        P1 = mk8("P1", DRT, dx.to_broadcast([128, T, 4]), OP.mult)
        P2 = mk8("P2", DCT, dy.to_broadcast([128, T, 4]), OP.mult)
        A = mk8("A", P1, P2, OP.add)
        Q1 = mk8("Q1", DRT, dy.to_broadcast([128, T, 4]), OP.mult)
        Q2 = mk8("Q2", DCT, dx.to_broadcast([128, T, 4]), OP.mult)
        B = mk8("B", Q2, Q1, OP.subtract)

        def ts8(name, in0, s1, s2, op0, op1=None):
            tl = t4(name)
            tv = tl[:].rearrange("p (t s) -> p t s", s=4)
            if op1 is None:
                nc.vector.tensor_scalar(tv, in0, s1, None, op0=op0)
            else:
                nc.vector.tensor_scalar(tv, in0, s1, s2, op0=op0, op1=op1)
            return tv

        def act8(name, in0, scale, bias):
            tl = t4(name)
            tv = tl[:].rearrange("p (t s) -> p t s", s=4)
            nc.scalar.activation(tv, in0, ACTF.Copy, bias=bias, scale=scale)
            return tv

        ZA = ts8("ZA", A, 0.0, None, OP.is_equal)
        ZB = ts8("ZB", B, 0.0, None, OP.is_equal)
        GF = ts8("GF", A, 0.5, None, OP.is_ge)
        GL = ts8("GL", B, 0.5, None, OP.is_ge)
        GR = ts8("GR", B, -0.5, None, OP.is_le)
        # u = 99 -/+ t on ACT (independent of the Q chains)
        UF = act8("UF", A, -1.0, 99.0)
        UL = act8("UL", B, -1.0, 99.0)
        UR = act8("UR", B, 1.0, 99.0)

        MX = tT("MX", 3)
        MXT = MX[:].rearrange("p (t c) -> p t c", c=3)
        specs = [
            (0, ZA, GL, UL),   # left: t=B, perp=-A
            (1, ZB, GF, UF),   # fwd:  t=A, perp=B
            (2, ZA, GR, UR),   # right: t=-B, perp=A
        ]
        for di, PZ, TG, U in specs:
            Q = mk8(f"Q{di}", PZ, TG, OP.mult)
            Qm = mk8(f"Qm{di}", Q, M25T, OP.mult)
            CC = mk8(f"CC{di}", U, Qm, OP.mult)
            nc.vector.tensor_reduce(
                MXT[:, :, di : di + 1], CC, axis=mybir.AxisListType.X, op=OP.max
            )

        # wall distances (left, fwd, right)
        DW = tT("DW", 3)
        DWT = DW[:].rearrange("p (t c) -> p t c", c=3)
        T1 = tT("T1")
        T1T = T1[:].rearrange("p (t c) -> p t c", c=1)
        nc.scalar.activation(T1T, hr, ACTF.Copy, bias=15.5, scale=-1.0)
        T2 = tT("T2")
        T2T = T2[:].rearrange("p (t c) -> p t c", c=1)
        nc.scalar.activation(T2T, hc, ACTF.Copy, bias=15.5, scale=-1.0)
        V1 = tT("V1")
        V1T = V1[:].rearrange("p (t c) -> p t c", c=1)
        nc.vector.tensor_tensor(V1T, dx, T1T, OP.mult)
        V2 = tT("V2")
        V2T = V2[:].rearrange("p (t c) -> p t c", c=1)
        nc.vector.tensor_tensor(V2T, dy, T2T, OP.mult)
        nc.vector.scalar_tensor_tensor(DWT[:, :, 1:2], V1T, 15.5, V2T, op0=OP.add, op1=OP.add)
        V3 = tT("V3")
        V3T = V3[:].rearrange("p (t c) -> p t c", c=1)
        nc.vector.tensor_tensor(V3T, dy, T1T, OP.mult)
        V4 = tT("V4")
        V4T = V4[:].rearrange("p (t c) -> p t c", c=1)
        nc.vector.tensor_tensor(V4T, dx, T2T, OP.mult)
        W0 = tT("W0")
        W0T = W0[:].rearrange("p (t c) -> p t c", c=1)
        nc.vector.scalar_tensor_tensor(W0T, V3T, -1.0, V4T, op0=OP.mult, op1=OP.add)
        nc.scalar.activation(DWT[:, :, 0:1], W0T, ACTF.Copy, bias=15.5, scale=1.0)
        nc.scalar.activation(DWT[:, :, 2:3], W0T, ACTF.Copy, bias=15.5, scale=-1.0)

        # free = min(dw, 99-MX) * [MX <= 97.5]
        TM = tT("TM", 3)
        TMT = TM[:].rearrange("p (t c) -> p t c", c=3)
        nc.scalar.activation(TMT, MXT, ACTF.Copy, bias=99.0, scale=-1.0)
        GT = tT("GT", 3)
        GTT = GT[:].rearrange("p (t c) -> p t c", c=3)
        nc.vector.tensor_scalar(GTT, MXT, 97.5, None, op0=OP.is_le)
        MN = tT("MN", 3)
        MNT = MN[:].rearrange("p (t c) -> p t c", c=3)
        nc.vector.tensor_tensor(MNT, DWT, TMT, OP.min)
        nc.vector.tensor_tensor(FEATT[:, :, 0:3], MNT, GTT, OP.mult)

        # ---- MLP ----
        W1V = WS[:, 0:100].rearrange("p (k j) -> p k j", j=20)
        B1V = WS[:, 100:120]
        W2V = WS[:, 120:180].rearrange("p (o j) -> p o j", j=20)
        B2V = WS[:, 180:183]

        H = tT("H", 20)
        HT = H[:].rearrange("p (t j) -> p t j", j=20)
        TMPK = tT("TMPK", 20)
        TMPKT = TMPK[:].rearrange("p (t j) -> p t j", j=20)
        for k in range(5):
            fk = FEATT[:, :, k : k + 1].to_broadcast([128, T, 20])
            wk = W1V[:, None, k, :].to_broadcast([128, T, 20])
            if k == 0:
                nc.vector.tensor_tensor(HT, fk, wk, OP.mult)
            else:
                nc.vector.tensor_tensor(TMPKT, fk, wk, OP.mult)
                nc.vector.tensor_tensor(HT, HT, TMPKT, OP.add)
        nc.vector.tensor_tensor(HT, HT, B1V[:, None, :].to_broadcast([128, T, 20]), OP.add)
        nc.scalar.activation(HT, HT, ACTF.Relu)

        PR2 = st.tile([128, T * 3 * 20], F32, tag="PR2", name="PR2")
        PR2T = PR2[:].rearrange("p (t o j) -> p t o j", o=3, j=20)
        nc.vector.tensor_tensor(
            PR2T,
            HT[:, :, None, :].to_broadcast([128, T, 3, 20]),
            W2V[:, None, :, :].to_broadcast([128, T, 3, 20]),
            OP.mult,
        )
        OT = tT("OT", 3)
        OTT = OT[:].rearrange("p (t o) -> p t o", o=3)
        nc.vector.reduce_sum(OTT, PR2T, axis=mybir.AxisListType.X)
        OT2 = tT("OT2", 3)
        OT2T = OT2[:].rearrange("p (t o) -> p t o", o=3)
        nc.vector.tensor_tensor(OT2T, OTT, B2V[:, None, :].to_broadcast([128, T, 3]), OP.add)

        nc.sync.dma_start(out=OUT, in_=OT2[:])



_NC_CACHE = {}


def _get_nc():
    if "nc" not in _NC_CACHE:
        nc = bacc.Bacc("TRN2", target_bir_lowering=False, debug=False,
                       num_devices=N_CORES)
        with tile.TileContext(nc) as tc:
            with tc.tile_pool(name="dram", bufs=1, space="DRAM") as dram:
                X = dram.tile([128, T, 1024], F32, kind="ExternalInput",
                              name="x", uniquify=False)
                G1TD = dram.tile([128, 8 * 128 * T], F32, kind="ExternalInput",
                                 name="g1t", uniquify=False)
                WALL = dram.tile([128, 183], F32, kind="ExternalInput",
                                 name="wall", uniquify=False)
                OUTD = dram.tile([128, T * 3], F32, kind="ExternalOutput",
                                 name="out", uniquify=False)
                build_dqn(tc, {"out": OUTD[:]},
                          {"x": X[:], "g1t": G1TD[:], "wall": WALL[:]}, T=T)
        nc.compile()
        _NC_CACHE["nc"] = nc
    return _NC_CACHE["nc"]


def _pack_weights(w1, b1, w2, b2):
    wall = np.zeros((183,), np.float32)
    wall[0:100] = np.asarray(w1, np.float32).T.reshape(-1)
    wall[100:120] = np.asarray(b1, np.float32)
    wall[120:180] = np.asarray(w2, np.float32).reshape(-1)
    wall[180:183] = np.asarray(b2, np.float32)
    return np.ascontiguousarray(np.broadcast_to(wall, (128, 183)))


def _prepare_in_maps(x, w1, b1, w2, b2):
    x = np.asarray(x, np.float32)
    assert x.shape == (B_FULL, 3, 32, 32), x.shape
    x01 = np.ascontiguousarray(x[:, :2].reshape(B_FULL, 2048))
    wall = _pack_weights(w1, b1, w2, b2)
    in_maps = []
    for c in range(N_CORES):
        sh = x01[c * B_CORE : (c + 1) * B_CORE]
        x0 = np.ascontiguousarray(sh[:, :1024].reshape(T, 128, 1024).transpose(1, 0, 2))
        g1 = sh[:, 1024:]
        g1r = g1.reshape(T, 128, 1024).transpose(2, 1, 0).reshape(1024, B_CORE)
        g1t = np.ascontiguousarray(
            g1r.reshape(8, 128, B_CORE).transpose(1, 0, 2).reshape(128, 8 * B_CORE))
        in_maps.append({"x": x0, "g1t": g1t, "wall": wall})
    return in_maps


def _assemble(results):
    parts = []
    for r in results:
        o = np.asarray(r["out"], np.float32)
        parts.append(o.reshape(128, T, 3).transpose(1, 0, 2).reshape(B_CORE, 3))
    return np.concatenate(parts, axis=0)


def run_spmd(x, w1, b1, w2, b2, trace=False):
    nc = _get_nc()
    in_maps = _prepare_in_maps(x, w1, b1, w2, b2)
    res = run_bass_kernel_spmd(nc, in_maps, core_ids=list(range(N_CORES)),
                               trace=trace)
    return _assemble(res.results), res


def kernel(x, w1, b1, w2, b2):
    out, _ = run_spmd(x, w1, b1, w2, b2, trace=False)
    return out


# revision 5
# speedup vs baseline: 1.1104x; 1.1104x over previous
"""Self-contained TRN2 Bass kernel for nn_DQN (snake feature extraction + tiny MLP).

kernel(**inputs) takes FULL inputs (x [8192,3,32,32] f32, w1, b1, w2, b2),
shards the batch dim across 8 NeuronCores (pure data parallel), runs a
Bass/Tile kernel via run_bass_kernel_spmd, returns the FULL (8192, 3) f32
output.

Device algorithm per core (1024 batches = 8 tiles of 128 partitions):
  Each batch occupies one partition lane; free dim holds [g0 grid | g1 grid]
  (2048 cells). Values: g0 in {-1 food, 0, 1 head, 2 obs}, g1 in {0, 1 prev}.
  Stream:  ab = Square(3*g0-1) in {16,1,4,25} [ACT]; max8+max_index give all
  special cells (<=4 obs, food, head) in one pass [DVE]; prev = sum(c*g1)
  via fused affine_mul_reduce [DVE]. Tiny per-batch stage reconstructs
  coordinates (bit ops on indices), builds the relative direction frame,
  raycasts against the <=4 obstacle coords + walls, rotates diff, applies
  the 5-20-3 MLP. Exact integer arithmetic in f32 throughout the features.
"""

import ml_dtypes
import numpy as np

import concourse.bacc as bacc
import concourse.mybir as mybir
import concourse.tile as tile
from concourse.bass_utils import run_bass_kernel_spmd

F32 = mybir.dt.float32
U16 = mybir.dt.uint16
I32 = mybir.dt.int32
BF16 = mybir.dt.bfloat16
OP = mybir.AluOpType
ACTF = mybir.ActivationFunctionType

N_CORES = 8
B_FULL = 8192
T = 8
B_CORE = 128 * T


def build_dqn(tc, outs, ins, T=8, bf16=True):
    nc = tc.nc
    X = ins["x"]          # [128, T, 1024] f32 DRAM (frame-0 grids)
    G1T = ins["g1t"]      # [128, 8*NB] bf16 DRAM (frame-1 transposed, [cellp, chunk, b'=p*8+t])
    CIDX = ins["cidx"]    # [128, 2048] bf16 DRAM: [c_hi = c>>2 | c_lo = c&3]
    EYE = ins["eye"]      # [128, 384] bf16 DRAM: [2048*I | 4*I | I]
    CSPL = ins["cspl"]    # [128, 16] bf16 DRAM: per chunk j cols (2j, 2j+1) = (cell>>2, cell&3)
    WALL = ins["wall"]    # [128, 183] f32 DRAM
    OUT = outs["out"]     # [128, T*3] f32 DRAM

    with (
        tc.tile_pool(name="io", bufs=3) as io,
        tc.tile_pool(name="abp", bufs=3) as abp,
        tc.tile_pool(name="tpl", bufs=3) as tplp,
        tc.tile_pool(name="st", bufs=1) as st,
        tc.tile_pool(name="ps", bufs=3, space="PSUM") as ps,
        tc.tile_pool(name="ps2", bufs=1, space="PSUM") as ps2,
        tc.tile_pool(name="dr", bufs=1, space="DRAM") as dr,
    ):
        WS = st.tile([128, 183], F32, name="WS")
        nc.sync.dma_start(out=WS, in_=WALL)
        CIDXS = st.tile([128, 2048], BF16, name="CIDXS")
        nc.sync.dma_start(out=CIDXS, in_=CIDX)
        EYES = st.tile([128, 384], BF16, name="EYES")
        nc.sync.dma_start(out=EYES, in_=EYE)
        CSPLS = st.tile([128, 16], BF16, name="CSPLS")
        nc.sync.dma_start(out=CSPLS, in_=CSPL)
        BN1 = st.tile([128, 1], F32, tag="BN1", name="BN1")
        nc.vector.memset(BN1[:], -1.0)

        VS = st.tile([128, T * 8], F32, name="VS")

        for t in range(T):
            xt = io.tile([128, 1024], F32, tag="xt", name=f"xt{t}")
            nc.sync.dma_start(out=xt, in_=X[:, t, :])
            ab = abp.tile([128, 1024], BF16, tag="ab", name=f"ab{t}")
            nc.scalar.activation(ab, xt, ACTF.Square, bias=BN1[:], scale=3.0)
            # T-plane = 2048*ab + c  (c = 4*c_hi + c_lo), exact ints in f32 PSUM
            pt = ps.tile([128, 1024], F32, tag="pt", name=f"pt{t}")
            for h in (0, 512):
                nc.tensor.matmul(pt[:, h : h + 512], lhsT=EYES[:, 0:128],
                                 rhs=ab[:, h : h + 512], start=True, stop=False)
                nc.tensor.matmul(pt[:, h : h + 512], lhsT=EYES[:, 128:256],
                                 rhs=CIDXS[:, h : h + 512], start=False, stop=False)
                nc.tensor.matmul(pt[:, h : h + 512], lhsT=EYES[:, 256:384],
                                 rhs=CIDXS[:, 1024 + h : 1024 + h + 512],
                                 start=False, stop=True)
            tpl = tplp.tile([128, 1024], F32, tag="tpl", name=f"tpl{t}")
            nc.scalar.copy(tpl, pt[:])
            nc.vector.max(out=VS[:, 8 * t : 8 * t + 8], in_=tpl)

        # ---- prev extraction on TensorE ----
        NB = 128 * T
        G1S = st.tile([128, 8 * NB], BF16, name="G1S")
        nc.sync.dma_start(out=G1S, in_=G1T)
        PM = ps2.tile([2, NB], F32, name="PM")
        for j in range(8):
            for h in range(0, NB, 512):
                he = min(h + 512, NB)
                nc.tensor.matmul(
                    PM[:, h:he],
                    lhsT=CSPLS[:, 2 * j : 2 * j + 2],
                    rhs=G1S[:, j * NB + h : j * NB + he],
                    start=(j == 0),
                    stop=(j == 7),
                )
        PRS = st.tile([2, NB], F32, name="PRS")
        nc.scalar.copy(PRS[:], PM[:])
        PRD = dr.tile([2 * NB], F32, name="PRD")
        nc.sync.dma_start(out=PRD, in_=PRS[:])
        PREVC2 = st.tile([128, 2 * T], F32, name="PREVC2")
        nc.sync.dma_start(
            out=PREVC2[:].rearrange("p (k t) -> p k t", k=2),
            in_=PRD[:].rearrange("(k p t) -> p k t", p=128, t=T))
        # prev flat idx = 4*hi + lo
        PREVC = st.tile([128, T], F32, name="PREVC")
        nc.vector.scalar_tensor_tensor(PREVC[:], PREVC2[:, 0:T], 4.0,
                                       PREVC2[:, T : 2 * T], op0=OP.mult, op1=OP.add)

        # ---------------- tiny stage ----------------
        def t8(name, dt=F32):
            return st.tile([128, T * 8], dt, tag=name, name=name)

        def tT(name, k=1, dt=F32):
            return st.tile([128, T * k], dt, tag=name, name=name)

        # decode top8 codes: v = 2048*band + c
        VI = t8("VI", I32)
        nc.vector.tensor_copy(VI[:], VS[:])
        VIT = VI[:].rearrange("p (t s) -> p t s", s=8)
        BND = t8("BND", I32)
        BNDT = BND[:].rearrange("p (t s) -> p t s", s=8)
        nc.vector.tensor_scalar(BNDT, VIT, 11, None, op0=OP.logical_shift_right)
        BNDF = t8("BNDF")
        nc.vector.tensor_copy(BNDF[:], BND[:])
        BNDFT = BNDF[:].rearrange("p (t s) -> p t s", s=8)

        M25 = st.tile([128, T * 4], F32, tag="M25", name="M25")
        M25T = M25[:].rearrange("p (t s) -> p t s", s=4)
        nc.vector.tensor_scalar(M25T, BNDFT[:, :, 0:4], 25.0, None, op0=OP.is_equal)

        # mask stack [128, T, 2, 8]: 0=food(16), 1=head(4)
        MST = st.tile([128, T * 2 * 8], F32, tag="MST", name="MST")
        MSTT = MST[:].rearrange("p (t m s) -> p t m s", m=2, s=8)
        nc.vector.tensor_scalar(MSTT[:, :, 0, :], BNDFT, 16.0, None, op0=OP.is_equal)
        nc.vector.tensor_scalar(MSTT[:, :, 1, :], BNDFT, 4.0, None, op0=OP.is_equal)

        # row/col per slot from idx bits of the code (low 11 bits = c)
        CP = st.tile([128, T * 2 * 8], F32, tag="CP", name="CP")
        CPT = CP[:].rearrange("p (t m s) -> p t m s", m=2, s=8)
        OCCI = t8("OCCI", I32)
        nc.vector.tensor_scalar(OCCI[:], VI[:], 31, None, op0=OP.bitwise_and)
        ORI = t8("ORI", I32)
        nc.vector.tensor_scalar(ORI[:], VI[:], 5, 31, op0=OP.logical_shift_right, op1=OP.bitwise_and)
        nc.vector.tensor_copy(CPT[:, :, 1, :], OCCI[:].rearrange("p (t s) -> p t s", s=8))
        nc.vector.tensor_copy(CPT[:, :, 0, :], ORI[:].rearrange("p (t s) -> p t s", s=8))

        # head/food coords: PROD[t,m,c,s] = CP[t,c,s]*MST[t,m,s] -> reduce s
        PROD = st.tile([128, T * 2 * 2 * 8], F32, tag="PROD", name="PROD")
        PRODT = PROD[:].rearrange("p (t m c s) -> p t m c s", m=2, c=2, s=8)
        nc.vector.tensor_tensor(
            PRODT,
            CPT[:, :, None, :, :].to_broadcast([128, T, 2, 2, 8]),
            MSTT[:, :, :, None, :].to_broadcast([128, T, 2, 2, 8]),
            OP.mult,
        )
        COORD = st.tile([128, T * 2 * 2], F32, tag="COORD", name="COORD")
        COORDT = COORD[:].rearrange("p (t m c) -> p t m c", m=2, c=2)
        nc.vector.reduce_sum(COORDT, PRODT, axis=mybir.AxisListType.X)
        # [:, :, 0, :] = (fr, fc); [:, :, 1, :] = (hr, hc)

        # prev coords from PREVC: int bits
        PRU = tT("PRU", 1, I32)
        nc.vector.tensor_copy(PRU[:], PREVC[:])
        PRB = st.tile([128, T * 2], I32, tag="PRB", name="PRB")
        PRBT = PRB[:].rearrange("p (t c) -> p t c", c=2)
        nc.vector.tensor_scalar(PRBT[:, :, 1:2], PRU[:].rearrange("p (t c) -> p t c", c=1), 31, None, op0=OP.bitwise_and)
        nc.vector.tensor_scalar(PRBT[:, :, 0:1], PRU[:].rearrange("p (t c) -> p t c", c=1), 5, 31, op0=OP.logical_shift_right, op1=OP.bitwise_and)
        PREVF = tT("PREVF", 2)
        PREVFT = PREVF[:].rearrange("p (t c) -> p t c", c=2)
        nc.vector.tensor_copy(PREVFT, PRBT)

        # d = head - prev ; diff = food - head
        D2 = tT("D2", 2)
        D2T = D2[:].rearrange("p (t c) -> p t c", c=2)
        nc.vector.tensor_tensor(D2T, COORDT[:, :, 1, :], PREVFT, OP.subtract)
        F2 = tT("F2", 2)
        F2T = F2[:].rearrange("p (t c) -> p t c", c=2)
        nc.vector.tensor_tensor(F2T, COORDT[:, :, 0, :], COORDT[:, :, 1, :], OP.subtract)

        dx = D2T[:, :, 0:1]
        dy = D2T[:, :, 1:2]
        hr = COORDT[:, :, 1, 0:1]
        hc = COORDT[:, :, 1, 1:2]

        FEAT = tT("FEAT", 5)
        FEATT = FEAT[:].rearrange("p (t c) -> p t c", c=5)
        RP = tT("RP", 4)
        RPT = RP[:].rearrange("p (t a b) -> p t a b", a=2, b=2)
        nc.vector.tensor_tensor(
            RPT,
            D2T[:, :, :, None].to_broadcast([128, T, 2, 2]),
            F2T[:, :, None, :].to_broadcast([128, T, 2, 2]),
            OP.mult,
        )
        nc.vector.tensor_tensor(FEATT[:, :, 3:4], RPT[:, :, 0, 0:1], RPT[:, :, 1, 1:2], OP.add)
        nc.vector.tensor_tensor(FEATT[:, :, 4:5], RPT[:, :, 0, 1:2], RPT[:, :, 1, 0:1], OP.subtract)

        # ---- rays (slots 0:4 only — obstacles always sort into the top 4) ----
        def t4(name, dt=F32):
            return st.tile([128, T * 4], dt, tag=name, name=name)

        DR = t4("DR")
        DRT = DR[:].rearrange("p (t s) -> p t s", s=4)
        nc.vector.tensor_tensor(DRT, CPT[:, :, 0, 0:4], hr.to_broadcast([128, T, 4]), OP.subtract)
        DC = t4("DC")
        DCT = DC[:].rearrange("p (t s) -> p t s", s=4)
        nc.vector.tensor_tensor(DCT, CPT[:, :, 1, 0:4], hc.to_broadcast([128, T, 4]), OP.subtract)

        def mk8(name, in0, in1, op):
            tl = t4(name)
            tv = tl[:].rearrange("p (t s) -> p t s", s=4)
            nc.vector.tensor_tensor(tv, in0, in1, op)
            return tv

        P1 = mk8("P1", DRT, dx.to_broadcast([128, T, 4]), OP.mult)
        P2 = mk8("P2", DCT, dy.to_broadcast([128, T, 4]), OP.mult)
        A = mk8("A", P1, P2, OP.add)
        Q1 = mk8("Q1", DRT, dy.to_broadcast([128, T, 4]), OP.mult)
        Q2 = mk8("Q2", DCT, dx.to_broadcast([128, T, 4]), OP.mult)
        B = mk8("B", Q2, Q1, OP.subtract)

        def ts8(name, in0, s1, s2, op0, op1=None):
            tl = t4(name)
            tv = tl[:].rearrange("p (t s) -> p t s", s=4)
            if op1 is None:
                nc.vector.tensor_scalar(tv, in0, s1, None, op0=op0)
            else:
                nc.vector.tensor_scalar(tv, in0, s1, s2, op0=op0, op1=op1)
            return tv

        def act8(name, in0, scale, bias):
            tl = t4(name)
            tv = tl[:].rearrange("p (t s) -> p t s", s=4)
            nc.scalar.activation(tv, in0, ACTF.Copy, bias=bias, scale=scale)
            return tv

        ZA = ts8("ZA", A, 0.0, None, OP.is_equal)
        ZB = ts8("ZB", B, 0.0, None, OP.is_equal)
        GF = ts8("GF", A, 0.5, None, OP.is_ge)
        GL = ts8("GL", B, 0.5, None, OP.is_ge)
        GR = ts8("GR", B, -0.5, None, OP.is_le)
        # u = 99 -/+ t on ACT (independent of the Q chains)
        UF = act8("UF", A, -1.0, 99.0)
        UL = act8("UL", B, -1.0, 99.0)
        UR = act8("UR", B, 1.0, 99.0)

        MX = tT("MX", 3)
        MXT = MX[:].rearrange("p (t c) -> p t c", c=3)
        specs = [
            (0, ZA, GL, UL),   # left: t=B, perp=-A
            (1, ZB, GF, UF),   # fwd:  t=A, perp=B
            (2, ZA, GR, UR),   # right: t=-B, perp=A
        ]
        for di, PZ, TG, U in specs:
            Q = mk8(f"Q{di}", PZ, TG, OP.mult)
            Qm = mk8(f"Qm{di}", Q, M25T, OP.mult)
            CC = mk8(f"CC{di}", U, Qm, OP.mult)
            nc.vector.tensor_reduce(
                MXT[:, :, di : di + 1], CC, axis=mybir.AxisListType.X, op=OP.max
            )

        # wall distances (left, fwd, right)
        DW = tT("DW", 3)
        DWT = DW[:].rearrange("p (t c) -> p t c", c=3)
        T1 = tT("T1")
        T1T = T1[:].rearrange("p (t c) -> p t c", c=1)
        nc.scalar.activation(T1T, hr, ACTF.Copy, bias=15.5, scale=-1.0)
        T2 = tT("T2")
        T2T = T2[:].rearrange("p (t c) -> p t c", c=1)
        nc.scalar.activation(T2T, hc, ACTF.Copy, bias=15.5, scale=-1.0)
        V1 = tT("V1")
        V1T = V1[:].rearrange("p (t c) -> p t c", c=1)
        nc.vector.tensor_tensor(V1T, dx, T1T, OP.mult)
        V2 = tT("V2")
        V2T = V2[:].rearrange("p (t c) -> p t c", c=1)
        nc.vector.tensor_tensor(V2T, dy, T2T, OP.mult)
        nc.vector.scalar_tensor_tensor(DWT[:, :, 1:2], V1T, 15.5, V2T, op0=OP.add, op1=OP.add)
        V3 = tT("V3")
        V3T = V3[:].rearrange("p (t c) -> p t c", c=1)
        nc.vector.tensor_tensor(V3T, dy, T1T, OP.mult)
        V4 = tT("V4")
        V4T = V4[:].rearrange("p (t c) -> p t c", c=1)
        nc.vector.tensor_tensor(V4T, dx, T2T, OP.mult)
        W0 = tT("W0")
        W0T = W0[:].rearrange("p (t c) -> p t c", c=1)
        nc.vector.scalar_tensor_tensor(W0T, V3T, -1.0, V4T, op0=OP.mult, op1=OP.add)
        nc.scalar.activation(DWT[:, :, 0:1], W0T, ACTF.Copy, bias=15.5, scale=1.0)
        nc.scalar.activation(DWT[:, :, 2:3], W0T, ACTF.Copy, bias=15.5, scale=-1.0)

        # free = min(dw, 99-MX) * [MX <= 97.5]
        TM = tT("TM", 3)
        TMT = TM[:].rearrange("p (t c) -> p t c", c=3)
        nc.scalar.activation(TMT, MXT, ACTF.Copy, bias=99.0, scale=-1.0)
        GT = tT("GT", 3)
        GTT = GT[:].rearrange("p (t c) -> p t c", c=3)
        nc.vector.tensor_scalar(GTT, MXT, 97.5, None, op0=OP.is_le)
        MN = tT("MN", 3)
        MNT = MN[:].rearrange("p (t c) -> p t c", c=3)
        nc.vector.tensor_tensor(MNT, DWT, TMT, OP.min)
        nc.vector.tensor_tensor(FEATT[:, :, 0:3], MNT, GTT, OP.mult)

        # ---- MLP ----
        W1V = WS[:, 0:100].rearrange("p (k j) -> p k j", j=20)
        B1V = WS[:, 100:120]
        W2V = WS[:, 120:180].rearrange("p (o j) -> p o j", j=20)
        B2V = WS[:, 180:183]

        H = tT("H", 20)
        HT = H[:].rearrange("p (t j) -> p t j", j=20)
        TMPK = tT("TMPK", 20)
        TMPKT = TMPK[:].rearrange("p (t j) -> p t j", j=20)
        for k in range(5):
            fk = FEATT[:, :, k : k + 1].to_broadcast([128, T, 20])
            wk = W1V[:, None, k, :].to_broadcast([128, T, 20])
            if k == 0:
                nc.vector.tensor_tensor(HT, fk, wk, OP.mult)
            else:
                nc.vector.tensor_tensor(TMPKT, fk, wk, OP.mult)
                nc.vector.tensor_tensor(HT, HT, TMPKT, OP.add)
        nc.vector.tensor_tensor(HT, HT, B1V[:, None, :].to_broadcast([128, T, 20]), OP.add)
        nc.scalar.activation(HT, HT, ACTF.Relu)

        PR2 = st.tile([128, T * 3 * 20], F32, tag="PR2", name="PR2")
        PR2T = PR2[:].rearrange("p (t o j) -> p t o j", o=3, j=20)
        nc.vector.tensor_tensor(
            PR2T,
            HT[:, :, None, :].to_broadcast([128, T, 3, 20]),
            W2V[:, None, :, :].to_broadcast([128, T, 3, 20]),
            OP.mult,
        )
        OT = tT("OT", 3)
        OTT = OT[:].rearrange("p (t o) -> p t o", o=3)
        nc.vector.reduce_sum(OTT, PR2T, axis=mybir.AxisListType.X)
        OT2 = tT("OT2", 3)
        OT2T = OT2[:].rearrange("p (t o) -> p t o", o=3)
        nc.vector.tensor_tensor(OT2T, OTT, B2V[:, None, :].to_broadcast([128, T, 3]), OP.add)

        nc.sync.dma_start(out=OUT, in_=OT2[:])



_NC_CACHE = {}


def _get_nc():
    if "nc" not in _NC_CACHE:
        nc = bacc.Bacc("TRN2", target_bir_lowering=False, debug=False,
                       num_devices=N_CORES)
        with tile.TileContext(nc) as tc:
            with tc.tile_pool(name="dram", bufs=1, space="DRAM") as dram:
                X = dram.tile([128, T, 1024], F32, kind="ExternalInput",
                              name="x", uniquify=False)
                G1TD = dram.tile([128, 8 * 128 * T], BF16, kind="ExternalInput",
                                 name="g1t", uniquify=False)
                CIDXD = dram.tile([128, 2048], BF16, kind="ExternalInput",
                                  name="cidx", uniquify=False)
                EYED = dram.tile([128, 384], BF16, kind="ExternalInput",
                                 name="eye", uniquify=False)
                CSPLD = dram.tile([128, 16], BF16, kind="ExternalInput",
                                  name="cspl", uniquify=False)
                WALL = dram.tile([128, 183], F32, kind="ExternalInput",
                                 name="wall", uniquify=False)
                OUTD = dram.tile([128, T * 3], F32, kind="ExternalOutput",
                                 name="out", uniquify=False)
                build_dqn(tc, {"out": OUTD[:]},
                          {"x": X[:], "g1t": G1TD[:], "cidx": CIDXD[:],
                           "eye": EYED[:], "cspl": CSPLD[:],
                           "wall": WALL[:]}, T=T)
        nc.compile()
        _NC_CACHE["nc"] = nc
    return _NC_CACHE["nc"]


def _pack_weights(w1, b1, w2, b2):
    wall = np.zeros((183,), np.float32)
    wall[0:100] = np.asarray(w1, np.float32).T.reshape(-1)
    wall[100:120] = np.asarray(b1, np.float32)
    wall[120:180] = np.asarray(w2, np.float32).reshape(-1)
    wall[180:183] = np.asarray(b2, np.float32)
    return np.ascontiguousarray(np.broadcast_to(wall, (128, 183)))


BF16NP = ml_dtypes.bfloat16


def _pack_consts():
    c = np.arange(1024)
    cidx = np.zeros((128, 2048), BF16NP)
    cidx[:, 0:1024] = (c >> 2).astype(BF16NP)[None, :]
    cidx[:, 1024:2048] = (c & 3).astype(BF16NP)[None, :]
    eye = np.zeros((128, 384), BF16NP)
    I = np.eye(128)
    eye[:, 0:128] = (2048 * I).astype(BF16NP)
    eye[:, 128:256] = (4 * I).astype(BF16NP)
    eye[:, 256:384] = I.astype(BF16NP)
    cspl = np.zeros((128, 16), BF16NP)
    p = np.arange(128)
    for j in range(8):
        cell = j * 128 + p
        cspl[:, 2 * j] = (cell >> 2).astype(BF16NP)
        cspl[:, 2 * j + 1] = (cell & 3).astype(BF16NP)
    return cidx, eye, cspl


def _prepare_in_maps(x, w1, b1, w2, b2):
    x = np.asarray(x, np.float32)
    assert x.shape == (B_FULL, 3, 32, 32), x.shape
    x01 = np.ascontiguousarray(x[:, :2].reshape(B_FULL, 2048))
    wall = _pack_weights(w1, b1, w2, b2)
    cidx, eye, cspl = _pack_consts()
    in_maps = []
    for c in range(N_CORES):
        sh = x01[c * B_CORE : (c + 1) * B_CORE]
        x0 = np.ascontiguousarray(sh[:, :1024].reshape(T, 128, 1024).transpose(1, 0, 2))
        g1 = sh[:, 1024:]
        g1r = g1.reshape(T, 128, 1024).transpose(2, 1, 0).reshape(1024, B_CORE)
        g1t = np.ascontiguousarray(
            g1r.reshape(8, 128, B_CORE).transpose(1, 0, 2)
            .reshape(128, 8 * B_CORE).astype(BF16NP))
        in_maps.append({"x": x0, "g1t": g1t, "cidx": cidx, "eye": eye,
                        "cspl": cspl, "wall": wall})
    return in_maps


def _assemble(results):
    parts = []
    for r in results:
        o = np.asarray(r["out"], np.float32)
        parts.append(o.reshape(128, T, 3).transpose(1, 0, 2).reshape(B_CORE, 3))
    return np.concatenate(parts, axis=0)


def run_spmd(x, w1, b1, w2, b2, trace=False):
    nc = _get_nc()
    in_maps = _prepare_in_maps(x, w1, b1, w2, b2)
    res = run_bass_kernel_spmd(nc, in_maps, core_ids=list(range(N_CORES)),
                               trace=trace)
    return _assemble(res.results), res


def kernel(x, w1, b1, w2, b2):
    out, _ = run_spmd(x, w1, b1, w2, b2, trace=False)
    return out
